# revision 39
# baseline (speedup 1.0000x reference)
"""MoE-LoRA layer kernel for Trainium2 (8 NeuronCores, data-parallel over tokens).

Computation (per reference):
  out = x @ W_base.T + b_base + scaling * sum_e combine[:,e] * (x @ A_e.T) @ B_e.T
  combine = renormalized top-2 softmax of router logits (= softmax over top-2 logits).

Sharding: 8192 tokens -> 1024 per core; all weights replicated. Layouts are
prepared host-side so device DMAs are contiguous (th = 512-token half):
  xt[p, th, kt, u]  = x[th*512+u, kt*128+p]        (bf16)
  x8[p, th, kf, u]  = x[th*512+u, (KB+kf)*128+p]   (fp8 copy, last KF k-tiles)
  wt[ot, p, kt, o]  = 64*W_base[ot*128+o, kt*128+p]   (bf16)
  w8[ot, p, kf, o]  = 64*W_base[ot*128+o, (KB+kf)*128+p]  (fp8)
  at[p, kt, er]     = A_all[er, kt*128+p]
  bt[er, o]         = 64*B_stack[e, o, r],  er = e*16+r
  rt[p, kt, e]      = W_router[e, kt*128+p]
  bias2[p, ot]      = b_base[ot*128+p]
Output: outt[ot, p, t] = out[t, ot*128+p]  (bf16).

The base GEMM is a split-K hybrid: KB k-tiles in bf16 (1 cycle/row) plus KF
k-tiles in fp8e4 DoubleRow (2 k-tiles/instruction, 2x rate). All base/LoRA
contributions carry a x64 scale so fp8 weights stay in e4m3 normal range;
the output activation applies scale=1/64 before the bias add. The window is
token-half-outer, with the softmax chain split so its matmul/transpose ops
never block the tensor queue mid-stream.
"""

import sys
import numpy as np
import ml_dtypes
from contextlib import ExitStack

try:
    import concourse.bass as bass
except ImportError:
    sys.path.insert(0, "/opt/trn_rl_repo")
    import concourse.bass as bass

import concourse.tile as tile
from concourse import bacc
from concourse import mybir
from concourse.bass import ts
from concourse.bass_utils import run_bass_kernel_spmd

F32 = mybir.dt.float32
BF16 = mybir.dt.bfloat16
FP8 = mybir.dt.float8e4
ALU = mybir.AluOpType
ACTF = mybir.ActivationFunctionType
AX = mybir.AxisListType
DR = mybir.MatmulPerfMode.DoubleRow
NPBF16 = ml_dtypes.bfloat16
NPFP8 = ml_dtypes.float8_e4m3

N_CORES = 8
D_IN = 4096
D_OUT = 4096
RANK = 16
NUM_EXPERTS = 8
ER = NUM_EXPERTS * RANK  # 128
TOP_K = 2
SCALING = 32.0 / RANK  # 2.0
KF = 8  # k-tiles of the base GEMM computed in fp8 DoubleRow (must be even)
WSCALE = 64.0  # power-of-2 scale keeping 64*W in e4m3 normal range


def build_nc(T=1024, KT=32, OT=32):
    """Build the per-core Bass kernel. T tokens, KT k-tiles (d_in=128*KT),
    OT out-tiles (d_out=128*OT). T must be a multiple of 512."""
    TH = T // 512  # token halves for 512-wide matmuls
    SPH = 512 // 128  # softmax subtiles per token half
    KB = KT - KF  # bf16 k-tiles
    nc = bacc.Bacc(None, target_bir_lowering=False, dynamic_dma_scratch_size=1024)

    xt = nc.dram_tensor("xt", [128, TH, KT, 512], BF16, kind="ExternalInput")
    x8t = nc.dram_tensor("x8t", [128, TH, KF, 512], FP8, kind="ExternalInput")
    wt = nc.dram_tensor("wt", [OT, 128, KB, 128], BF16, kind="ExternalInput")
    w8t = nc.dram_tensor("w8t", [OT, 128, KF, 128], FP8, kind="ExternalInput")
    at = nc.dram_tensor("at", [128, KB, ER], BF16, kind="ExternalInput")
    at8 = nc.dram_tensor("at8", [128, KF, ER], FP8, kind="ExternalInput")
    bt = nc.dram_tensor("bt", [ER, 128 * OT], BF16, kind="ExternalInput")
    rt = nc.dram_tensor("rt", [128, KT, NUM_EXPERTS], BF16, kind="ExternalInput")
    bias2 = nc.dram_tensor("bias2", [128, OT], F32, kind="ExternalInput")
    id2 = nc.dram_tensor("id2", [128, 128], F32, kind="ExternalInput")
    expand = nc.dram_tensor("expand", [NUM_EXPERTS, ER], BF16, kind="ExternalInput")
    outt = nc.dram_tensor("outt", [OT, 128, T], BF16, kind="ExternalOutput")
    INV = float(1.0 / WSCALE)

    with tile.TileContext(nc) as tc, ExitStack() as ctx:
        const = ctx.enter_context(tc.tile_pool(name="const", bufs=1))
        xpool = ctx.enter_context(tc.tile_pool(name="xp", bufs=1))
        wpool = ctx.enter_context(tc.tile_pool(name="wp", bufs=4))
        w8pool = ctx.enter_context(tc.tile_pool(name="w8p", bufs=4))
        btp = ctx.enter_context(tc.tile_pool(name="btp", bufs=3))
        hpool = ctx.enter_context(tc.tile_pool(name="hp", bufs=1))
        smt = ctx.enter_context(tc.tile_pool(name="smt", bufs=4))
        opool = ctx.enter_context(tc.tile_pool(name="op", bufs=4))
        pmain = ctx.enter_context(
            tc.tile_pool(name="pmain", bufs=max(3 * TH, 4), space="PSUM")
        )
        psmall = ctx.enter_context(tc.tile_pool(name="psm", bufs=2, space="PSUM"))
        E = NUM_EXPERTS

        # ---- DMAs: the x stream owns the scalar HWDGE queue (its own
        # descriptor generator); weights/consts flow on the sync queue ----
        at_s = wpool.tile([128, KB, 128], BF16, tag="w")
        w0_s = wpool.tile([128, KB, 128], BF16, tag="w")
        x_s = xpool.tile([128, TH, KT, 512], BF16)
        x8_s = xpool.tile([128, TH, KF, 512], FP8)

        # x on the scalar queue, token-half-major to match the window order;
        # first two k-tiles land as singles, the rest as 4-ktile chunks
        # (4KB contiguous per partition). The sync queue is reserved for the
        # weight slivers the window consumes in lockstep with x — routing x
        # chunks there delays the slivers past their need times.
        for th in range(TH):
            for kt in range(min(2, KT)):
                nc.scalar.dma_start(x_s[:, th, kt, :], xt[:, th, kt, :])
            k0 = 2
            while k0 < KT:
                k1 = min(k0 + 4, KT)
                nc.scalar.dma_start(x_s[:, th, k0:k1, :], xt[:, th, k0:k1, :])
                k0 = k1
            nc.scalar.dma_start(x8_s[:, th], x8t[:, th])

        # weights on sync queue, paced so early k-tiles don't steal the x
        # stream's HBM share: tiny head slivers, consts, then the rest
        rt_s = const.tile([128, KT, E], BF16)
        at8_s = const.tile([128, KF, ER], FP8)
        w08_s = const.tile([128, KF, 128], FP8)
        nc.sync.dma_start(at_s[:, 0:2, :], at[:, 0:2, :])
        nc.sync.dma_start(w0_s[:, 0:2, :], wt[0, :, 0:2, :])
        nc.sync.dma_start(at_s[:, 2:8, :], at[:, 2:8, :])
        nc.sync.dma_start(w0_s[:, 2:8, :], wt[0, :, 2:8, :])
        nc.sync.dma_start(rt_s, rt[:])

        id_s = const.tile([128, 128], F32)
        nc.sync.dma_start(id_s, id2[:])
        bias_s = const.tile([128, OT], F32)
        nc.sync.dma_start(bias_s, bias2[:])
        exp_s = const.tile([E, ER], BF16)
        nc.sync.dma_start(exp_s, expand[:])
        b0_s = const.tile([ER, 128], BF16)
        nc.sync.dma_start(b0_s, bt[:, 0:128])

        mid = (8 + KB) // 2
        nc.sync.dma_start(at_s[:, 8:mid, :], at[:, 8:mid, :])
        nc.sync.dma_start(w0_s[:, 8:mid, :], wt[0, :, 8:mid, :])
        nc.sync.dma_start(at_s[:, mid:KB, :], at[:, mid:KB, :])
        nc.sync.dma_start(w0_s[:, mid:KB, :], wt[0, :, mid:KB, :])
        nc.sync.dma_start(at8_s, at8[:])
        nc.sync.dma_start(w08_s, w8t[0, :, :, :])

        # ---- window: A-proj + router + base(ot=0) share the x stream,
        # token-half-outer so softmax overlaps the second half ----
        ph = [pmain.tile([128, 512], F32, tag="pm", name=f"ph{i}") for i in range(TH)]
        plT = [pmain.tile([E, 512], F32, tag="pm", name=f"plT{i}") for i in range(TH)]
        po0 = [pmain.tile([128, 512], F32, tag="pm", name=f"po0{i}") for i in range(TH)]

        h_s = hpool.tile([128, T], BF16)
        hw_r = h_s  # weighted in place; rhs of the B matmuls
        lT = hpool.tile([E, T], F32)
        l_t = {}

        def softmax_a(s_i):
            """transpose this 128-token chunk's logits out of PSUM; cheap on
            the tensor queue (waits only on the lT copy)."""
            ptl = psmall.tile([128, E], F32, tag="ps", name="ptl")
            nc.tensor.transpose(ptl, lT[:, ts(s_i, 128)], id_s[:E, :E])
            l = smt.tile([128, E], F32, name="l")
            nc.vector.tensor_copy(l, ptl)
            l_t[s_i] = l

        def softmax_b(s_i):
            """combine weights for tokens [s_i*128, (s_i+1)*128), weighted
            into hw_r in place. The DVE chain runs behind whatever matmul
            stream precedes this in program order."""
            l = l_t.pop(s_i)
            m1 = smt.tile([128, 1], F32)
            nc.vector.reduce_max(m1, l, axis=AX.X)
            lm = smt.tile([128, E], F32)  # logits - max  (<= 0, ==0 at argmax)
            nc.vector.tensor_scalar(lm, l, m1, None, op0=ALU.subtract)
            isz = smt.tile([128, E], F32)
            nc.vector.tensor_scalar(isz, lm, 0.0, None, op0=ALU.is_equal)
            pen = smt.tile([128, E], F32)
            nc.vector.tensor_scalar(pen, isz, -1e30, None, op0=ALU.mult)
            msk = smt.tile([128, E], F32)
            nc.vector.tensor_tensor(msk, lm, pen, op=ALU.add)
            m2 = smt.tile([128, 1], F32)  # second max, relative to m1
            nc.vector.reduce_max(m2, msk, axis=AX.X)
            e_t = smt.tile([128, E], F32)
            nc.scalar.activation(e_t, lm, ACTF.Exp)
            e2 = smt.tile([128, 1], F32)
            nc.scalar.activation(e2, m2, ACTF.Exp)
            den = smt.tile([128, 1], F32)
            nc.vector.tensor_scalar(den, e2, 1.0, None, op0=ALU.add)
            inv = smt.tile([128, 1], F32)
            nc.vector.reciprocal(inv, den)
            ge = smt.tile([128, E], F32)  # top-2 membership mask
            nc.vector.tensor_scalar(ge, lm, m2, None, op0=ALU.is_ge)
            cmb = smt.tile([128, E], F32)
            nc.vector.tensor_tensor(cmb, e_t, ge, op=ALU.mult)
            cmb2 = smt.tile([128, E], F32)
            nc.vector.tensor_scalar(cmb2, cmb, inv, None, op0=ALU.mult)
            pt = psmall.tile([E, 128], F32, tag="ps", name="pt")
            nc.tensor.transpose(pt, cmb2, id_s)
            ct = smt.tile([E, 128], BF16)
            nc.vector.tensor_copy(ct, pt)
            pc = psmall.tile([128, 128], F32, tag="ps", name="pc")
            nc.tensor.matmul(pc, exp_s, ct, start=True, stop=True)
            nc.vector.tensor_tensor(
                hw_r[:, ts(s_i, 128)], h_s[:, ts(s_i, 128)], pc, op=ALU.mult
            )

        def emit_window_half(th):
            for kt in range(KB):
                st = kt == 0
                xc = x_s[:, th, kt, :]
                nc.tensor.matmul(ph[th], at_s[:, kt, :], xc, start=st, stop=False)
                nc.tensor.matmul(plT[th], rt_s[:, kt, :], xc, start=st, stop=False)
                nc.tensor.matmul(po0[th], w0_s[:, kt, :], xc, start=st, stop=False)
            # fp8 tail: A and base0 as DoubleRow pairs; router stays bf16
            for j in range(KF // 2):
                kf = 2 * j
                last = j == KF // 2 - 1
                for dk in range(2):
                    kt = KB + kf + dk
                    nc.tensor.matmul(
                        plT[th], rt_s[:, kt, :], x_s[:, th, kt, :],
                        start=False, stop=(last and dk == 1),
                    )
                nc.tensor.matmul(
                    ph[th], at8_s[:, kf : kf + 2, :], x8_s[:, th, kf : kf + 2, :],
                    start=False, stop=last, perf_mode=DR,
                )
                nc.tensor.matmul(
                    po0[th], w08_s[:, kf : kf + 2, :], x8_s[:, th, kf : kf + 2, :],
                    start=False, stop=False, perf_mode=DR,
                )
            nc.vector.tensor_copy(h_s[:, ts(th, 512)], ph[th])
            nc.vector.tensor_copy(lT[:, ts(th, 512)], plT[th])
            for s_i in range(th * SPH, (th + 1) * SPH):
                softmax_a(s_i)

        emit_window_half(0)
        if TH > 1:
            emit_window_half(1)
        for s_i in range(0, SPH):
            softmax_b(s_i)
        if TH > 1:
            for s_i in range(SPH, 2 * SPH):
                softmax_b(s_i)

        def load_w(ot, split=False):
            w_s = wpool.tile([128, KB, 128], BF16, tag="w")
            if split:
                # first half unblocks the o-tile's first matmuls; the second
                # trails in behind them, keeping those bytes out of the
                # DMA-bound window period
                half = KB // 2
                nc.sync.dma_start(w_s[:, 0:half, :], wt[ot, :, 0:half, :])
                nc.sync.dma_start(w_s[:, half:KB, :], wt[ot, :, half:KB, :])
            else:
                nc.sync.dma_start(w_s, wt[ot, :, :, :])
            w8_s = w8pool.tile([128, KF, 128], FP8, tag="w8")
            nc.sync.dma_start(w8_s, w8t[ot, :, :, :])
            b_sl = btp.tile([ER, 128], BF16)
            nc.sync.dma_start(b_sl, bt[:, ts(ot, 128)])
            return w_s, w8_s, b_sl

        def emit_base(ot, w_s, w8_s, b_sl=None):
            # kt outer / th inner: consecutive matmuls share the stationary
            # weight tile; the last KF k-tiles run as fp8 DoubleRow pairs.
            # When b_sl is given (hw_r already final), the LoRA B matmul
            # leads the accumulation group instead of trailing it.
            pos = [
                pmain.tile([128, 512], F32, tag="pm", name=f"po_{ot}_{th}")
                for th in range(TH)
            ]
            if b_sl is not None:
                for th in range(TH):
                    nc.tensor.matmul(
                        pos[th], b_sl, hw_r[:, ts(th, 512)], start=True, stop=False
                    )
            for kt in range(KB):
                for th in range(TH):
                    nc.tensor.matmul(
                        pos[th],
                        w_s[:, kt, :],
                        x_s[:, th, kt, :],
                        start=(kt == 0 and b_sl is None),
                        stop=False,
                    )
            for kf in range(0, KF, 2):
                last = b_sl is not None and kf == KF - 2
                for th in range(TH):
                    nc.tensor.matmul(
                        pos[th],
                        w8_s[:, kf : kf + 2, :],
                        x8_s[:, th, kf : kf + 2, :],
                        start=False,
                        stop=last,
                        perf_mode=DR,
                    )
            return pos

        def emit_tail(ot, pos, b_sl):
            for th in range(TH):
                if b_sl is not None:
                    nc.tensor.matmul(
                        pos[th], b_sl, hw_r[:, ts(th, 512)], start=False, stop=True
                    )
                o_t = opool.tile([128, 512], BF16, tag="o_t", name=f"ot_{ot}_{th}")
                nc.scalar.activation(
                    o_t, pos[th], ACTF.Identity, bias=bias_s[:, ot : ot + 1], scale=INV
                )
                nc.scalar.dma_start(outt[ot, :, ts(th, 512)], o_t)

        def emit_tail_last(ot, pos):
            # drain the final o-tile in 256-token chunks, alternating the
            # scalar and vector engines so the post-matmul latency shrinks
            for th in range(TH):
                for q in range(2):
                    o_q = opool.tile(
                        [128, 256], BF16, tag="o_t", name=f"oq_{th}_{q}"
                    )
                    src = pos[th][:, ts(q, 256)]
                    if q == 0:
                        nc.scalar.activation(
                            o_q, src, ACTF.Identity,
                            bias=bias_s[:, ot : ot + 1], scale=INV,
                        )
                    else:
                        nc.vector.tensor_scalar(
                            o_q, src, INV, bias_s[:, ot : ot + 1],
                            op0=ALU.mult, op1=ALU.add,
                        )
                    eng = nc.scalar
                    eng.dma_start(outt[ot, :, ts(th * 2 + q, 256)], o_q)

        first = min(1, OT - 1)
        w1, w81, b1 = load_w(first, split=True)
        pos1 = emit_base(first, w1, w81)
        emit_tail(first, pos1, b1)

        # ---- ot=0 LoRA term accumulated into the held PSUM group ----
        for th in range(TH):
            nc.tensor.matmul(
                po0[th], b0_s, hw_r[:, ts(th, 512)], start=False, stop=True
            )
            o_t = opool.tile([128, 512], BF16, name=f"oo0_{th}", tag="o_t")
            nc.scalar.activation(
                o_t, po0[th], ACTF.Identity, bias=bias_s[:, 0:1], scale=INV
            )
            nc.scalar.dma_start(outt[0, :, ts(th, 512)], o_t)

        # ---- remaining o-tiles, processed in pairs with the two fp8
        # DoubleRow sections adjacent: the PE pays the fp8 stationary
        # double-load once per pair instead of once per o-tile. hw_r is
        # final here, so the LoRA B matmul can sit anywhere in each group.
        def emit_pair(a, b):
            w_a, w8_a, b_a = load_w(a)
            w_b, w8_b, b_b = load_w(b)
            pos_a = [
                pmain.tile([128, 512], F32, tag="pm", name=f"po_{a}_{th}")
                for th in range(TH)
            ]
            pos_b = [
                pmain.tile([128, 512], F32, tag="pm", name=f"po_{b}_{th}")
                for th in range(TH)
            ]
            for th in range(TH):
                nc.tensor.matmul(
                    pos_a[th], b_a, hw_r[:, ts(th, 512)], start=True, stop=False
                )
            for kt in range(KB):
                for th in range(TH):
                    nc.tensor.matmul(
                        pos_a[th], w_a[:, kt, :], x_s[:, th, kt, :],
                        start=False, stop=False,
                    )
            for kf in range(0, KF, 2):
                for th in range(TH):
                    nc.tensor.matmul(
                        pos_a[th], w8_a[:, kf : kf + 2, :],
                        x8_s[:, th, kf : kf + 2, :],
                        start=False, stop=(kf == KF - 2), perf_mode=DR,
                    )
            for kf in range(0, KF, 2):
                for th in range(TH):
                    nc.tensor.matmul(
                        pos_b[th], w8_b[:, kf : kf + 2, :],
                        x8_s[:, th, kf : kf + 2, :],
                        start=(kf == 0), stop=False, perf_mode=DR,
                    )
            for th in range(TH):
                nc.tensor.matmul(
                    pos_b[th], b_b, hw_r[:, ts(th, 512)], start=False, stop=False
                )
            for kt in range(KB):
                for th in range(TH):
                    nc.tensor.matmul(
                        pos_b[th], w_b[:, kt, :], x_s[:, th, kt, :],
                        start=False, stop=(kt == KB - 1),
                    )
            return pos_a, pos_b

        ot = 2
        while ot < OT:
            if ot + 1 < OT:
                pos_a, pos_b = emit_pair(ot, ot + 1)
                emit_tail(ot, pos_a, None)
                if ot + 1 == OT - 1:
                    emit_tail_last(ot + 1, pos_b)
                else:
                    emit_tail(ot + 1, pos_b, None)
                ot += 2
            else:
                w_s, w8_s, b_sl = load_w(ot)
                pos = emit_base(ot, w_s, w8_s, b_sl)
                emit_tail_last(ot, pos)
                ot += 1

    nc.compile()
    return nc


def prep_shared(W_base, b_base, W_router, A_stack, B_stack, KT=32, OT=32):
    """Host-side layout prep for the replicated weights."""
    D = KT * 128
    O = OT * 128
    KB = KT - KF
    W_base = np.asarray(W_base, dtype=np.float32) * np.float32(WSCALE)
    w4 = W_base.reshape(OT, 128, KT, 128).transpose(0, 3, 2, 1)  # [ot, p, kt, o]
    wt = np.ascontiguousarray(w4[:, :, :KB, :]).astype(NPBF16)
    w8 = np.ascontiguousarray(w4[:, :, KB:, :]).astype(NPFP8)
    # A carries the same x64 scale (compensated via the expand matrix)
    A_all = np.asarray(A_stack, dtype=np.float32).reshape(ER, D) * np.float32(WSCALE)
    a3 = A_all.reshape(ER, KT, 128).transpose(2, 1, 0)  # [p, kt, er]
    at = np.ascontiguousarray(a3[:, :KB, :]).astype(NPBF16)
    at8 = np.ascontiguousarray(a3[:, KB:, :]).astype(NPFP8)
    bt = np.ascontiguousarray(
        np.asarray(B_stack, dtype=np.float32).transpose(0, 2, 1).reshape(ER, O)
        * np.float32(WSCALE)
    ).astype(NPBF16)
    rtT = np.asarray(W_router, dtype=np.float32).T  # [D, E]
    rt = np.ascontiguousarray(
        rtT.reshape(KT, 128, NUM_EXPERTS).transpose(1, 0, 2)
    ).astype(NPBF16)
    bias2 = np.ascontiguousarray(np.asarray(b_base, dtype=np.float32).reshape(OT, 128).T)
    id2 = np.eye(128, dtype=np.float32)
    expand = np.repeat(
        np.eye(NUM_EXPERTS, dtype=np.float32) * np.float32(SCALING / WSCALE),
        RANK, axis=1,
    ).astype(NPBF16)
    return dict(
        wt=wt, w8t=w8, at=at, at8=at8, bt=bt, rt=rt, bias2=bias2, id2=id2,
        expand=expand,
    )


def make_in_maps(x, W_base, b_base, W_router, A_stack, B_stack, T=1024, KT=32, OT=32):
    shared = prep_shared(W_base, b_base, W_router, A_stack, B_stack, KT, OT)
    KB = KT - KF
    TH = T // 512
    xf = np.asarray(x, dtype=np.float32).reshape(-1, D_IN)
    in_maps = []
    for c in range(N_CORES):
        x_c = xf[c * T : (c + 1) * T]  # [T, D]
        # [p, th, kt, u]
        x4 = x_c.reshape(TH, 512, KT, 128).transpose(3, 0, 2, 1)
        xt = np.ascontiguousarray(x4).astype(NPBF16)
        x8 = np.ascontiguousarray(x4[:, :, KB:, :]).astype(NPFP8)
        m = dict(shared)
        m["xt"] = xt
        m["x8t"] = x8
        in_maps.append(m)
    return in_maps


_NC_CACHE = {}


def _get_nc(T, KT, OT):
    key = (T, KT, OT)
    if key not in _NC_CACHE:
        _NC_CACHE[key] = build_nc(T, KT, OT)
    return _NC_CACHE[key]


def kernel(x, W_base, b_base, W_router, A_stack, B_stack):
    x = np.asarray(x, dtype=np.float32)
    orig_shape = x.shape
    N = x.reshape(-1, D_IN).shape[0]
    T = N // N_CORES
    KT = D_IN // 128
    OT = D_OUT // 128

    nc = _get_nc(T, KT, OT)
    in_maps = make_in_maps(x, W_base, b_base, W_router, A_stack, B_stack, T, KT, OT)

    res = run_bass_kernel_spmd(nc, in_maps, core_ids=list(range(N_CORES)))
    out = np.empty((N, D_OUT), dtype=np.float32)
    for c in range(N_CORES):
        outt = res.results[c]["outt"]  # [OT, 128, T] bf16
        out[c * T : (c + 1) * T] = (
            outt.astype(np.float32).transpose(2, 0, 1).reshape(T, D_OUT)
        )
    return out.reshape(orig_shape[:-1] + (D_OUT,))
